# revision 1
# baseline (speedup 1.0000x reference)
"""Trainium2 Bass kernel for the two-template sparse cross-modal attention module.

Sharding: data-parallel over batch B=32 across 8 NeuronCores (4 samples/core).
Each sample carries two modality streams (v, i) that must be co-resident
because search tokens attend to the template keys of BOTH modalities.

Per-core program (per sample s, streams st in {v, i}):
  1. QK^T projection in transposed layout: QKT[1536, 384] = qkv_w[0:1536] @ x.T
     (lhsT = qkv_w.T chunks, rhs = x.T chunks) -> per-head Q.T, K.T [64, tok].
  2. V projection in natural layout: V[384, 768] = x @ qkv_w[1536:].T
     (lhsT = x.T chunks, rhs = qkv_w.T[:, 1536:]) stored with a ones column
     per head ([tok, 65]) so the AV matmul also accumulates the softmax
     denominator l as an extra output row.
  3. Attention per head, scores transposed (S.T[k, q] = K Q.T, contract Dh):
     softmax without max-subtraction (scores are O(1); exp is safe), the
     denominator comes from the ones column, normalization by 1/l applied via
     a gpsimd partition_broadcast of recip_l + one DVE multiply.
     Search queries attend to [own k_mt, other-modality k_mt, own k_s].
  4. Output projection from the transposed attention output (lhsT = O.T
     chunks, rhs = proj_w.T) -> natural-layout Y [384, 768], bias added via a
     K=1 ones matmul, contiguous DMA out.
"""

import numpy as np

for _p in ("/opt/trn_rl_repo", "/root/.axon_site/_ro/trn_rl_repo"):
    import os
    import sys

    if os.path.isdir(_p) and _p not in sys.path:
        sys.path.append(_p)

B = 32
N_CORES = 8
SAMPLES = 4  # per core
C = 768
NTOK = 384
H = 12
DH = 64
MT = 128  # template tokens
CCH = C // 128  # 6 contraction chunks
MCH = 12  # QK row chunks (1536/128)
TCH = NTOK // 128  # 3 token chunks
SCALE = DH ** (-0.5)

_PROG_CACHE = {}


def _build_program(mm_f32r, es_bf16, with_bias=True):
    import concourse.bass as bass  # noqa: F401
    import concourse.tile as tile
    from concourse import bacc, mybir

    f32 = mybir.dt.float32
    f32r = mybir.dt.float32r
    bf16 = mybir.dt.bfloat16
    mdt = f32r if mm_f32r else f32
    esdt = bf16 if es_bf16 else mdt
    Act = mybir.ActivationFunctionType

    nc = bacc.Bacc(None, target_bir_lowering=False)
    if mm_f32r or es_bf16:
        import contextlib

        _lp = nc.allow_low_precision(reason="fp32r/bf16 matmul inputs, fp32 PSUM accumulation")
    else:
        import contextlib

        _lp = contextlib.nullcontext()
    _lp.__enter__()

    xt_d = nc.dram_tensor("xt", [2 * SAMPLES, C, NTOK], f32, kind="ExternalInput")
    qkvw_d = nc.dram_tensor("qkvwT", [C, 3 * C], f32, kind="ExternalInput")
    projw_d = nc.dram_tensor("projwT", [C, C], f32, kind="ExternalInput")
    bias_d = nc.dram_tensor("bias", [1, C], f32, kind="ExternalInput")
    y_d = nc.dram_tensor("y", [2 * SAMPLES, NTOK, C], f32, kind="ExternalOutput")

    dma_in = nc.gpsimd if mm_f32r else nc.sync

    with tile.TileContext(nc) as tc:
        with (
            tc.tile_pool(name="consts", bufs=1) as consts,
            tc.tile_pool(name="xtp", bufs=2) as xtp,
            tc.tile_pool(name="qktp", bufs=1) as qktp,
            tc.tile_pool(name="v1p", bufs=1) as v1p,
            tc.tile_pool(name="otp", bufs=1) as otp,
            tc.tile_pool(name="esp", bufs=4) as esp,
            tc.tile_pool(name="rlp", bufs=2) as rlp,
            tc.tile_pool(name="rlbp", bufs=2) as rlbp,
            tc.tile_pool(name="yp", bufs=3) as yp,
            tc.tile_pool(name="pap", bufs=3, space="PSUM") as pap,
            tc.tile_pool(name="psp", bufs=3, space="PSUM") as psp,
            tc.tile_pool(name="pop", bufs=2, space="PSUM") as pop,
        ):
            qkvw_sb = consts.tile([128, CCH, 3 * C], mdt)
            projw_sb = consts.tile([128, CCH, C], mdt)
            bias_sb = consts.tile([1, C], mdt)
            ones_row = consts.tile([1, 128], mdt)
            ones_f32 = consts.tile([128, 128], f32)
            nc.vector.memset(ones_f32, 1.0)
            for c in range(CCH):
                dma_in.dma_start(
                    out=qkvw_sb[:, c, :], in_=qkvw_d[c * 128 : (c + 1) * 128, :]
                )
                dma_in.dma_start(
                    out=projw_sb[:, c, :], in_=projw_d[c * 128 : (c + 1) * 128, :]
                )
            dma_in.dma_start(out=bias_sb, in_=bias_d[:, :])
            nc.vector.tensor_copy(out=ones_row, in_=ones_f32[0:1, 0:128])

            for s in range(SAMPLES):
                xt_sb = xtp.tile([128, CCH, 2, NTOK], mdt, tag="xt")
                for st in range(2):
                    for c in range(CCH):
                        dma_in.dma_start(
                            out=xt_sb[:, c, st, :],
                            in_=xt_d[2 * s + st, c * 128 : (c + 1) * 128, :],
                        )

                # ---- phase 1: QK^T (transposed layout) ----
                qkt_sb = qktp.tile([128, MCH, 2, NTOK], mdt, tag="qkt")
                for m in range(MCH):
                    for st in range(2):
                        pq = pap.tile([128, NTOK], f32, tag="pa")
                        for c in range(CCH):
                            nc.tensor.matmul(
                                pq,
                                qkvw_sb[:, c, m * 128 : (m + 1) * 128],
                                xt_sb[:, c, st, :],
                                start=(c == 0),
                                stop=(c == CCH - 1),
                            )
                        nc.scalar.activation(
                            out=qkt_sb[:, m, st, :], in_=pq, func=Act.Copy
                        )

                # ---- phase 2: V (natural layout, with ones column) ----
                v1_sb = v1p.tile([128, TCH, 2, H, 65], mdt, tag="v1")
                for t in range(TCH):
                    for st in range(2):
                        for n in range(2):
                            pv = pap.tile([128, NTOK], f32, tag="pa")
                            for c in range(CCH):
                                nc.tensor.matmul(
                                    pv,
                                    xt_sb[:, c, st, t * 128 : (t + 1) * 128],
                                    qkvw_sb[:, c, 2 * C + n * NTOK : 2 * C + (n + 1) * NTOK],
                                    start=(c == 0),
                                    stop=(c == CCH - 1),
                                )
                            nc.vector.tensor_copy(
                                out=v1_sb[:, t, st, 6 * n : 6 * n + 6, 0:64],
                                in_=pv.rearrange("p (h d) -> p h d", h=6),
                            )
                nc.vector.tensor_copy(
                    out=v1_sb[:, :, :, :, 64:65],
                    in_=ones_f32[:, 0:72].rearrange(
                        "p (t s h) -> p t s h", t=TCH, s=2
                    ).unsqueeze(4),
                )

                # ---- phase 3: attention ----
                # Heads are processed in even/odd pairs: their Q.T/K.T slices
                # sit at partition bases 0 and 64, so the two K=64 score
                # matmuls target distinct PE row-groups; emitting them
                # back-to-back lets the hardware run them concurrently.
                ot_sb = otp.tile([128, CCH, 2, NTOK], mdt, tag="ot")
                for st in range(2):
                    for hp in range(6):
                        po_pair = [
                            pop.tile([65, NTOK], f32, tag="po", name=f"po_{s}_{st}_{hp}_{i}")
                            for i in range(2)
                        ]
                        # per chunk: S-mm pair (adjacent), exps, AV pair
                        for ci in range(4):
                            es_pair = []
                            ps_pair = []
                            for i in range(2):
                                h = 2 * hp + i
                                ro = i * 64
                                qT = qkt_sb[ro : ro + 64, hp, st, :]
                                kT = qkt_sb[ro : ro + 64, 6 + hp, st, :]
                                kTo = qkt_sb[ro : ro + 64, 6 + hp, 1 - st, :]
                                if ci == 0:
                                    lk, rq, nq = kT[:, 0:MT], qT, NTOK
                                elif ci == 1:
                                    lk, rq, nq = kTo[:, 0:MT], qT[:, MT:], 256
                                else:
                                    j = ci - 2
                                    lk = kT[:, MT + j * 128 : MT + (j + 1) * 128]
                                    rq, nq = qT[:, MT:], 256
                                psc = psp.tile(
                                    [128, nq], f32, tag="ps", name=f"ps_{s}_{st}_{hp}_{ci}_{i}"
                                )
                                nc.tensor.matmul(psc, lk, rq, start=True, stop=True)
                                ps_pair.append(psc)
                            for i in range(2):
                                ei = esp.tile(
                                    [128, nq], esdt, tag="es", name=f"es_{s}_{st}_{hp}_{ci}_{i}"
                                )
                                nc.scalar.activation(
                                    ei, ps_pair[i], Act.Exp, scale=SCALE
                                )
                                es_pair.append(ei)
                            for i in range(2):
                                h = 2 * hp + i
                                vst = (1 - st) if ci == 1 else st
                                vt = 0 if ci < 2 else ci - 1
                                dst = po_pair[i] if ci == 0 else po_pair[i][:, MT:]
                                nc.tensor.matmul(
                                    dst,
                                    v1_sb[:, vt, vst, h, :],
                                    es_pair[i],
                                    start=(ci == 0),
                                    stop=(ci == 3),
                                )
                        for i in range(2):
                            h = 2 * hp + i
                            ro = i * 64
                            po = po_pair[i]
                            rl = rlp.tile([1, NTOK], f32, tag="rl", name=f"rl_{s}_{st}_{hp}_{i}")
                            nc.vector.reciprocal(out=rl, in_=po[64:65, :])
                            rlb = rlbp.tile([64, NTOK], f32, tag="rlb", name=f"rlb_{s}_{st}_{hp}_{i}")
                            nc.gpsimd.partition_broadcast(rlb, rl)
                            nc.vector.tensor_mul(
                                ot_sb[ro : ro + 64, hp, st, :], po[0:64, :], rlb
                            )

                # ---- phase 4: output projection ----
                for st in range(2):
                    for t in range(TCH):
                        y_sb = yp.tile([128, C], f32, tag="y")
                        for n2 in range(2):
                            py = pap.tile([128, NTOK], f32, tag="pa")
                            for c in range(CCH):
                                nc.tensor.matmul(
                                    py,
                                    ot_sb[:, c, st, t * 128 : (t + 1) * 128],
                                    projw_sb[:, c, n2 * NTOK : (n2 + 1) * NTOK],
                                    start=(c == 0),
                                    stop=(not with_bias and c == CCH - 1),
                                )
                            if with_bias:
                                nc.tensor.matmul(
                                    py,
                                    ones_row[0:1, :],
                                    bias_sb[0:1, n2 * NTOK : (n2 + 1) * NTOK],
                                    start=False,
                                    stop=True,
                                )
                            nc.vector.tensor_copy(
                                out=y_sb[:, n2 * NTOK : (n2 + 1) * NTOK], in_=py
                            )
                        nc.sync.dma_start(
                            out=y_d[2 * s + st, t * 128 : (t + 1) * 128, :], in_=y_sb
                        )

    _lp.__exit__(None, None, None)
    nc.compile()
    return nc


def _get_program(mm_f32r=True, es_bf16=False, with_bias=True):
    key = (mm_f32r, es_bf16, with_bias)
    if key not in _PROG_CACHE:
        _PROG_CACHE[key] = _build_program(mm_f32r, es_bf16, with_bias)
    return _PROG_CACHE[key]


def _prep_in_maps(x_v, x_i, qkv_w, proj_w, proj_b):
    qkvwT = np.ascontiguousarray(qkv_w.T.astype(np.float32))
    projwT = np.ascontiguousarray(proj_w.T.astype(np.float32))
    bias = np.ascontiguousarray(proj_b.astype(np.float32).reshape(1, C))
    in_maps = []
    for core in range(N_CORES):
        sl = slice(core * SAMPLES, (core + 1) * SAMPLES)
        # interleave: stream 2s = v-sample, 2s+1 = i-sample, transposed to [C, NTOK]
        xs = np.empty((2 * SAMPLES, C, NTOK), np.float32)
        xs[0::2] = np.asarray(x_v[sl]).transpose(0, 2, 1)
        xs[1::2] = np.asarray(x_i[sl]).transpose(0, 2, 1)
        in_maps.append(
            {
                "xt": np.ascontiguousarray(xs),
                "qkvwT": qkvwT,
                "projwT": projwT,
                "bias": bias,
            }
        )
    return in_maps


def kernel(x_v, x_i, qkv_w, proj_w, proj_b, t_h, t_w, s_h, s_w, num_heads):
    from concourse.bass_utils import run_bass_kernel_spmd

    x_v = np.asarray(x_v, np.float32)
    x_i = np.asarray(x_i, np.float32)
    nc = _get_program(with_bias=bool(np.any(np.asarray(proj_b))))
    in_maps = _prep_in_maps(x_v, x_i, qkv_w, proj_w, proj_b)
    res = run_bass_kernel_spmd(nc, in_maps, list(range(N_CORES)))
    out_v = np.empty((B, NTOK, C), np.float32)
    out_i = np.empty((B, NTOK, C), np.float32)
    for core in range(N_CORES):
        y = res.results[core]["y"]
        sl = slice(core * SAMPLES, (core + 1) * SAMPLES)
        out_v[sl] = y[0::2]
        out_i[sl] = y[1::2]
    return out_v, out_i



# revision 34
# speedup vs baseline: 1.6414x; 1.6414x over previous
"""Trainium2 Bass kernel for the two-template sparse cross-modal attention module.

Sharding: data-parallel over batch B=32 across 8 NeuronCores (4 samples/core).
Each sample carries two modality streams (v, i) that must be co-resident
because search tokens attend to the template keys of BOTH modalities.

All matmul operands are bf16 (1 PE cycle/row regardless of free size); all
data I/O is bf16 with host-side conversion.

Per-core program (per sample s, streams st in {v, i}):
  A. QK^T in transposed layout: QKT[1536, 384] = qkv_w[0:1536] @ x.T
     (per-head Q.T, K.T as [64, tok] partition rows).
  B. V in natural layout [tok, 768] with a ones column per head
     ([tok, 65]) so the AV matmul also emits the softmax denominator l
     as a per-partition column.
  C. Attention per head with scores transposed (S.T[k, q] = K Q.T) but AV
     in NATURAL layout: o[q, 65] = sum_k es[k, q].T-matmul v1[k, 65].
     Moving dim of AV is 65 (vs 384 transposed) -> half the PE rows.  The
     denominator lands per-partition, so normalization is one [128,1]
     reciprocal + one broadcast tensor_mul per head pair (no partition
     broadcast).  Scores per head pack into 3 PSUM banks; exp runs as 5
     Act instructions per head pair ([384]x2, [512]x3).
  D. O natural -> O.T via PE transpose (identity matmul, [128,128] tiles).
  E. Transposed projection Y.T[cout, tok] = projw.T-chunks @ O.T with the
     bias folded into the PSUM->SBUF copy as a per-partition tensor_scalar
     add.  Y.T is DMAed out in bf16; the host transposes back.

Emission is software-pipelined: attention of sample s (Act/exp-bound) is
interleaved in program order with its own transpose+projection (as soon as
each head pair is normalized) and with phases A/B of sample s+1 (PE-bound),
so neither the PE nor the Act engine waits on the other.  Weight DMAs are
split so the first matmul starts after ~2 transfers.
"""

import numpy as np

for _p in ("/opt/trn_rl_repo", "/root/.axon_site/_ro/trn_rl_repo"):
    import os
    import sys

    if os.path.isdir(_p) and _p not in sys.path:
        sys.path.append(_p)

B = 32
N_CORES = 8
SAMPLES = 4  # per core
C = 768
NTOK = 384
H = 12
DH = 64
MT = 128  # template tokens
CCH = C // 128  # 6 contraction chunks
MCH = 12  # QK row chunks (1536/128)
TCH = NTOK // 128  # 3 token chunks
WSCALE = 64.0
SCALE = DH ** (-0.5)
SSCALE = SCALE / (WSCALE * WSCALE)

_PROG_CACHE = {}


def _build_program():
    import concourse.bass as bass  # noqa: F401
    import concourse.tile as tile
    from concourse import bacc, mybir
    from concourse.masks import make_identity

    f32 = mybir.dt.float32
    bf16 = mybir.dt.bfloat16
    fp8 = mybir.dt.float8e4
    DR = mybir.MatmulPerfMode.DoubleRow
    Act = mybir.ActivationFunctionType

    nc = bacc.Bacc(None, target_bir_lowering=False)
    _lp = nc.allow_low_precision(reason="fp8/bf16 matmul inputs, fp32 PSUM accumulation")
    _lp.__enter__()

    # x and qkv_w ship as error-compensated fp8 pairs (hi + residual, both
    # e4m3, weights pre-scaled by WSCALE): three DoubleRow matmuls
    # (hi*hi + hi*lo + lo*hi) give bf16-grade accuracy at 2x the fp8 rate.
    # The 64x weight scale rides through scores (folded into the exp scale)
    # and through V (the denominator ones-column is 64 as well).
    xt_d = [
        nc.dram_tensor(f"xt{p}", [2 * SAMPLES, 128, CCH, NTOK], fp8, kind="ExternalInput")
        for p in "hl"
    ]
    qkvw_d = [
        nc.dram_tensor(f"qkvwT{p}", [128, CCH, 3 * C], fp8, kind="ExternalInput")
        for p in "hl"
    ]
    projw_d = nc.dram_tensor("projwT", [128, CCH, C], bf16, kind="ExternalInput")
    bias_d = nc.dram_tensor("bias", [128, CCH], f32, kind="ExternalInput")
    y_d = nc.dram_tensor("y", [2 * SAMPLES, 128, CCH, NTOK], bf16, kind="ExternalOutput")

    with tile.TileContext(nc) as tc:
        with (
            tc.tile_pool(name="consts", bufs=1) as consts,
            tc.tile_pool(name="xtp", bufs=2) as xtp,
            tc.tile_pool(name="qktp", bufs=2) as qktp,
            tc.tile_pool(name="v1p", bufs=2) as v1p,
            tc.tile_pool(name="osbp", bufs=2) as osbp,
            tc.tile_pool(name="otTp", bufs=2) as otTp,
            tc.tile_pool(name="yp", bufs=2) as yp,
            tc.tile_pool(name="esp", bufs=4) as esp,
            tc.tile_pool(name="rp", bufs=4) as rp,
            tc.tile_pool(name="pap", bufs=3, space="PSUM") as pap,
            tc.tile_pool(name="pscap", bufs=1, space="PSUM") as pscap,
            tc.tile_pool(name="pscbp", bufs=1, space="PSUM") as pscbp,
            tc.tile_pool(name="psccp", bufs=1, space="PSUM") as psccp,
            tc.tile_pool(name="popp", bufs=2, space="PSUM") as popp,
        ):
            qkvw_sb = [consts.tile([128, CCH, 3 * C], fp8, name=f"qkvw{p}") for p in "hl"]
            projw_sb = consts.tile([128, CCH, C], bf16)
            bias_sb = consts.tile([128, CCH], f32)
            ident = consts.tile([128, 128], bf16)
            make_identity(nc, ident)

            xt_t = [None] * SAMPLES
            qkt_t = [None] * SAMPLES
            v1_t = [None] * SAMPLES
            osb_t = [None] * SAMPLES
            otT_t = [None] * SAMPLES
            y_t = [[None, None] for _ in range(SAMPLES)]
            acopy_ctr = [0]  # round-robin counter for A-copy engine split

            def dma_const_units():
                # qkv weights split so phase A can start after ~2 DMAs
                for j in range(4):
                    for p in range(2):
                        yield lambda j=j, p=p: nc.sync.dma_start(
                            out=qkvw_sb[p][:, :, j * 384 : (j + 1) * 384],
                            in_=qkvw_d[p][:, :, j * 384 : (j + 1) * 384],
                        )
                for p in range(2):
                    yield lambda p=p: nc.sync.dma_start(
                        out=qkvw_sb[p][:, :, 1536:2304], in_=qkvw_d[p][:, :, 1536:2304]
                    )
                yield lambda: nc.sync.dma_start(out=projw_sb, in_=projw_d[:, :, :])
                yield lambda: nc.sync.dma_start(out=bias_sb, in_=bias_d[:, :])

            def dma_in_units(s):
                xt_t[s] = [
                    xtp.tile([128, CCH, 2, NTOK], fp8, tag=f"xt{p}", name=f"xt{p}_{s}")
                    for p in "hl"
                ]
                for st in range(2):
                    for p in range(2):
                        yield lambda st=st, p=p: nc.sync.dma_start(
                            out=xt_t[s][p][:, :, st, :], in_=xt_d[p][2 * s + st]
                        )

            def psum_copy(out, in_):
                # GPSIMD cannot touch PSUM on hardware: split the PSUM->SBUF
                # copies between the Act engine (1 in 6) and the DVE
                if acopy_ctr[0] % 6 == 0:
                    nc.scalar.activation(out, in_, Act.Copy)
                else:
                    nc.vector.tensor_copy(out=out, in_=in_)
                acopy_ctr[0] += 1

            # compensated-fp8 DoubleRow contraction: hi*hi + hi*lo + lo*hi
            HL = ((0, 0), (0, 1), (1, 0))

            def a_unit(s, st, m):
                pq = pap.tile([128, NTOK], f32, tag="pa", name=f"pa_a{s}_{st}_{m}")
                for wp, xp in HL:
                    for c2 in range(CCH // 2):
                        nc.tensor.matmul(
                            pq,
                            qkvw_sb[wp][:, 2 * c2 : 2 * c2 + 2, m * 128 : (m + 1) * 128],
                            xt_t[s][xp][:, 2 * c2 : 2 * c2 + 2, st, :],
                            start=((wp, xp) == HL[0] and c2 == 0),
                            stop=((wp, xp) == HL[-1] and c2 == CCH // 2 - 1),
                            perf_mode=DR,
                        )
                psum_copy(qkt_t[s][:, m, st, :], pq)

            def b_unit(s, st, t, n):
                pv = pap.tile([128, NTOK], f32, tag="pa", name=f"pa_b{s}_{st}_{t}_{n}")
                for xp, wp in HL:
                    for c2 in range(CCH // 2):
                        nc.tensor.matmul(
                            pv,
                            xt_t[s][xp][
                                :, 2 * c2 : 2 * c2 + 2, st, t * 128 : (t + 1) * 128
                            ],
                            qkvw_sb[wp][
                                :, 2 * c2 : 2 * c2 + 2,
                                2 * C + n * NTOK : 2 * C + (n + 1) * NTOK,
                            ],
                            start=((xp, wp) == HL[0] and c2 == 0),
                            stop=((xp, wp) == HL[-1] and c2 == CCH // 2 - 1),
                            perf_mode=DR,
                        )
                psum_copy(
                    v1_t[s][:, t, st, 6 * n : 6 * n + 6, 0:64],
                    pv.rearrange("p (h d) -> p h d", h=6),
                )
                if t == 0 and n == 0:
                    # 64 = WSCALE: the AV denominator column must carry the
                    # same scale as the (pre-scaled) V values
                    nc.vector.memset(v1_t[s][:, :, st, :, 64:65], 64.0)

            def ab_units(s):
                """Phase A (QK^T transposed) + phase B (V natural) for sample s,
                ordered so attention pair (st, hp) is ready as early as possible:
                A chunks hp-major (q then k, both streams), B chunks n-major."""
                qkt_t[s] = qktp.tile(
                    [128, MCH, 2, NTOK], bf16, tag="qkt", name=f"qkt_{s}"
                )
                v1_t[s] = v1p.tile(
                    [128, TCH, 2, H, 65], bf16, tag="v1", name=f"v1_{s}"
                )
                for m in range(MCH):
                    for st in range(2):
                        yield lambda st=st, m=m: a_unit(s, st, m)
                for n in range(2):
                    for t in range(TCH):
                        for st in range(2):
                            yield lambda st=st, t=t, n=n: b_unit(s, st, t, n)

            # Attention is emitted as a one-head software pipeline: the S
            # matmuls + exps of head h+1 are interleaved with the AV matmuls
            # of head h, so the PE->Act->PE loop of a single head never sits
            # on the critical path and the Act queue stays continuously fed.
            ht = {}  # (s, st, h) -> dict of live tiles

            def s_ab(s, st, h):
                """Scores for the own-mt (slot 0, exp A) and own-search
                (slot 1, exp B) keys, plus their exps."""
                qkt = qkt_t[s]
                # separate tiles per score slot: dependency tracking is
                # tile-granular, so a shared tile would serialize the next
                # head's score matmuls behind ALL of this head's exps
                pscA = pscap.tile([128, NTOK], f32, tag="pscA", name=f"pscA_{s}_{st}_{h}")
                pscB = pscbp.tile([128, 512], f32, tag="pscB", name=f"pscB_{s}_{st}_{h}")
                es = esp.tile([128, 3, 512], bf16, tag="es", name=f"es_{s}_{st}_{h}")
                ht[(s, st, h)] = {"pscA": pscA, "pscB": pscB, "es": es}
                ro = (h % 2) * 64
                hp = h // 2
                qT = qkt[ro : ro + 64, hp, st, :]
                kT = qkt[ro : ro + 64, 6 + hp, st, :]
                nc.tensor.matmul(pscA, kT[:, 0:MT], qT)
                nc.tensor.matmul(pscB[:, 0:256], kT[:, MT : MT + 128], qT[:, MT:])
                nc.tensor.matmul(pscB[:, 256:512], kT[:, MT + 128 :], qT[:, MT:])
                nc.scalar.activation(es[:, 0, 0:NTOK], pscA, Act.Exp, scale=SSCALE)
                nc.scalar.activation(es[:, 1, :], pscB, Act.Exp, scale=SSCALE)

            def s_c(s, st, h):
                """Scores for the other-modality template keys (slot 2) + exp."""
                qkt = qkt_t[s]
                t = ht[(s, st, h)]
                pscC = psccp.tile([128, 256], f32, tag="pscC", name=f"pscC_{s}_{st}_{h}")
                ro = (h % 2) * 64
                hp = h // 2
                qT = qkt[ro : ro + 64, hp, st, :]
                kTo = qkt[ro : ro + 64, 6 + hp, 1 - st, :]
                nc.tensor.matmul(pscC, kTo[:, 0:MT], qT[:, MT:])
                nc.scalar.activation(
                    t["es"][:, 2, 0:256], pscC, Act.Exp, scale=SSCALE
                )

            def av_full(s, st, h):
                """All AV matmuls for head h.  Runs a full pipeline period
                after the head's exps, so nothing here waits on the Act
                engine.  Each search q-chunk's accumulation group runs to
                completion before the next opens: start_tensor_calc lazily
                zeroes the whole PSUM tile, so interleaving open groups in
                one tile destroys the earlier group's partial sums.
                """
                v1 = v1_t[s]
                t = ht[(s, st, h)]
                es = t["es"]
                # 96-f32 stride keeps every matmul PSUM dst 16B-aligned
                po = popp.tile([128, TCH, 96], f32, tag="po", name=f"po_{s}_{st}_{h}")
                t["po"] = po
                # mt queries: attend own-mt keys only (closed group)
                nc.tensor.matmul(po[:, 0, 0:65], es[:, 0, 0:MT], v1[:, 0, st, h, :])
                for u in (1, 2):
                    qo = (u - 1) * 128
                    dst = po[:, u, 0:65]
                    nc.tensor.matmul(
                        dst, es[:, 0, MT + qo : MT + qo + 128],
                        v1[:, 0, st, h, :], start=True, stop=False,
                    )
                    nc.tensor.matmul(
                        dst, es[:, 1, qo : qo + 128],
                        v1[:, 1, st, h, :], start=False, stop=False,
                    )
                    nc.tensor.matmul(
                        dst, es[:, 1, 256 + qo : 256 + qo + 128],
                        v1[:, 2, st, h, :], start=False, stop=False,
                    )
                    nc.tensor.matmul(
                        dst, es[:, 2, qo : qo + 128],
                        v1[:, 0, 1 - st, h, :], start=False, stop=True,
                    )

            def av_finish(s, st, h):
                """Reciprocal of the denominator column + broadcast normalize."""
                if osb_t[s] is None:
                    osb_t[s] = osbp.tile(
                        [128, TCH, 2, C], bf16, tag="osb", name=f"osb_{s}"
                    )
                po = ht.pop((s, st, h))["po"]
                rl = rp.tile([128, TCH], f32, tag="rl", name=f"rl_{s}_{st}_{h}")
                nc.vector.reciprocal(out=rl, in_=po[:, :, 64:65])
                nc.vector.tensor_mul(
                    osb_t[s][:, :, st, h * 64 : (h + 1) * 64],
                    po[:, :, 0:64],
                    rl[:, :, None].broadcast_to([128, TCH, 64]),
                )

            def d_unit(s, st, cc):
                if otT_t[s] is None:
                    otT_t[s] = otTp.tile(
                        [128, CCH, 2, NTOK], bf16, tag="otT", name=f"otT_{s}"
                    )
                pt = pap.tile([128, NTOK], bf16, tag="pa", name=f"pa_d{s}_{st}_{cc}")
                for u in range(TCH):
                    nc.tensor.transpose(
                        pt[:, u * 128 : (u + 1) * 128],
                        osb_t[s][:, u, st, cc * 128 : (cc + 1) * 128],
                        ident,
                    )
                nc.vector.tensor_copy(out=otT_t[s][:, cc, st, :], in_=pt)

            def e_unit(s, st, m2):
                if y_t[s][st] is None:
                    y_t[s][st] = yp.tile(
                        [128, CCH, NTOK], bf16, tag="y", name=f"y_{s}_{st}"
                    )
                py = pap.tile([128, NTOK], f32, tag="pa", name=f"pa_e{s}_{st}_{m2}")
                for c in range(CCH):
                    nc.tensor.matmul(
                        py,
                        projw_sb[:, c, m2 * 128 : (m2 + 1) * 128],
                        otT_t[s][:, c, st, :],
                        start=(c == 0),
                        stop=(c == CCH - 1),
                    )
                nc.vector.tensor_scalar_add(
                    y_t[s][st][:, m2, :], py, bias_sb[:, m2 : m2 + 1]
                )
                # two half-DMAs per stream so the last one drains faster
                if m2 == CCH // 2 - 1 or m2 == CCH - 1:
                    half = m2 // (CCH // 2)
                    sl = slice(half * (CCH // 2), (half + 1) * (CCH // 2))
                    nc.sync.dma_start(
                        out=y_d[2 * s + st, :, sl, :], in_=y_t[s][st][:, sl, :]
                    )

            def de_units(s):
                for st in range(2):
                    for cc in range(CCH):
                        yield lambda st=st, cc=cc: d_unit(s, st, cc)
                    for m2 in range(CCH):
                        yield lambda st=st, m2=m2: e_unit(s, st, m2)

            # ---- software-pipelined emission ----
            # One continuous stream: attention heads of every sample in
            # sequence, with a single global filler queue (phases A/B of the
            # next sample, D/E of the current one as their inputs retire).
            # Filler is consumed at splice points inside each head, paced by
            # estimated PE time, and spills across sample boundaries.
            from collections import deque

            fill_q = deque()
            spent = [0.0]  # estimated PE-ns of filler consumed

            def splice_upto(tgt_ns):
                while fill_q and spent[0] < tgt_ns:
                    run_one()

            consts_dma = list(dma_const_units())
            first_in = list(dma_in_units(0))
            # first xt stream (hi+lo), first weight pieces, then the rest,
            # so A(0) starts after four small transfers
            first_in[0]()
            first_in[1]()
            consts_dma[0]()
            consts_dma[1]()
            first_in[2]()
            first_in[3]()
            for u in consts_dma[2:]:
                u()
            for u in ab_units(0):
                u()

            # filler pacing: per-head PE-ns of filler, slightly below the
            # production rate so a backlog accumulates for the last sample
            PER_HEAD = 1500.0
            tgt_base = [0.0]
            appended = [0]  # items ever appended to fill_q
            ran = [0]  # items ever consumed

            def run_one():
                cost, u, then = fill_q.popleft()
                u()
                ran[0] += 1
                spent[0] += cost
                if then:
                    for item in then:
                        fill_q.append(item)
                        appended[0] += 1

            def push(cost, u, then=None):
                fill_q.append((cost, u, then))
                appended[0] += 1

            def after_finish(s, st, h):
                if h % 2 == 1:
                    hp = h // 2
                    then = None
                    if hp == 5:
                        then = [
                            (960.0,
                             (lambda s=s, st=st, m2=m2: e_unit(s, st, m2)),
                             None)
                            for m2 in range(CCH)
                        ]
                    push(200.0, (lambda s=s, st=st, hp=hp: d_unit(s, st, hp)),
                         then)

            markers = {}
            flat = [
                (s, st, h)
                for s in range(SAMPLES)
                for st in range(2)
                for h in range(H)
            ]
            per_head = [PER_HEAD]
            prev = [None]

            def step(cur):
                """One pipeline step: AV of the previous head wrapped around
                S+exp of the current one, filler spliced at the two points
                where the PE would otherwise wait on the Act engine."""
                s, st, h = cur
                if st == 0 and h == 0:
                    if s + 1 < SAMPLES:
                        for u in dma_in_units(s + 1):
                            push(0.0, u)
                        for u in ab_units(s + 1):
                            push(720.0, u)
                        # A/B of s+1 must be fully emitted before the first
                        # head of s+1 (the in-order PE queue would otherwise
                        # invert the qkt/v1 dependencies)
                        markers[s + 1] = appended[0]
                    else:
                        # final sample: spread the backlog + its own D/E
                        # evenly over the remaining heads
                        left = sum(c for c, _, _ in fill_q) + 2 * H * 600.0
                        per_head[0] = left / (2 * H)
                    if s > 0:
                        while ran[0] < markers[s]:
                            run_one()
                p = prev[0]
                if p is not None:
                    # AV of the previous head: all three exps it needs
                    # completed during the previous period, so none of these
                    # matmuls ever wait on the Act engine
                    av_full(*p)
                s_ab(s, st, h)
                s_c(s, st, h)
                splice_upto(tgt_base[0] + 0.55 * per_head[0])
                if p is not None:
                    av_finish(*p)
                    after_finish(*p)
                splice_upto(tgt_base[0] + per_head[0])
                tgt_base[0] += per_head[0]
                prev[0] = cur

            for cur in flat:
                step(cur)
            av_full(*prev[0])
            av_finish(*prev[0])
            after_finish(*prev[0])
            while fill_q:
                run_one()

    _lp.__exit__(None, None, None)
    nc.compile()
    return nc


def _get_program():
    if "prog" not in _PROG_CACHE:
        _PROG_CACHE["prog"] = _build_program()
    return _PROG_CACHE["prog"]


def _to_bf16(a):
    import ml_dtypes

    return np.ascontiguousarray(a.astype(ml_dtypes.bfloat16))


def _to_fp8_pair(a):
    import ml_dtypes

    f8 = ml_dtypes.float8_e4m3
    hi = a.astype(f8)
    lo = (a - hi.astype(np.float32)).astype(f8)
    return np.ascontiguousarray(hi), np.ascontiguousarray(lo)


def _prep_in_maps(x_v, x_i, qkv_w, proj_w, proj_b):
    # weights: [out, in] -> transposed [in, out] -> [128, CCH, out] chunked
    qkvwT = np.asarray(qkv_w, np.float32).T.reshape(CCH, 128, 3 * C).transpose(1, 0, 2)
    projwT = np.asarray(proj_w, np.float32).T.reshape(CCH, 128, C).transpose(1, 0, 2)
    bias = np.ascontiguousarray(
        np.asarray(proj_b, np.float32).reshape(CCH, 128).T
    )
    qkvwTh, qkvwTl = _to_fp8_pair(qkvwT * WSCALE)
    projwT = _to_bf16(projwT)
    in_maps = []
    for core in range(N_CORES):
        sl = slice(core * SAMPLES, (core + 1) * SAMPLES)
        # streams interleaved: 2s = v-sample, 2s+1 = i-sample;
        # layout [128, CCH, NTOK]: partition p, chunk c -> channel c*128+p
        xs = np.empty((2 * SAMPLES, 128, CCH, NTOK), np.float32)
        xs[0::2] = (
            np.asarray(x_v[sl], np.float32)
            .transpose(0, 2, 1)
            .reshape(SAMPLES, CCH, 128, NTOK)
            .transpose(0, 2, 1, 3)
        )
        xs[1::2] = (
            np.asarray(x_i[sl], np.float32)
            .transpose(0, 2, 1)
            .reshape(SAMPLES, CCH, 128, NTOK)
            .transpose(0, 2, 1, 3)
        )
        xth, xtl = _to_fp8_pair(xs)
        in_maps.append(
            {
                "xth": xth,
                "xtl": xtl,
                "qkvwTh": qkvwTh,
                "qkvwTl": qkvwTl,
                "projwT": projwT,
                "bias": bias,
            }
        )
    return in_maps


def _decode_out(res):
    out_v = np.empty((B, NTOK, C), np.float32)
    out_i = np.empty((B, NTOK, C), np.float32)
    for core in range(N_CORES):
        y = np.asarray(res.results[core]["y"], dtype=np.float32)
        # [2S, 128, CCH, NTOK] -> [2S, CCH*128 = C, NTOK] -> [2S, NTOK, C]
        y = y.transpose(0, 2, 1, 3).reshape(2 * SAMPLES, C, NTOK).transpose(0, 2, 1)
        sl = slice(core * SAMPLES, (core + 1) * SAMPLES)
        out_v[sl] = y[0::2]
        out_i[sl] = y[1::2]
    return out_v, out_i


def kernel(x_v, x_i, qkv_w, proj_w, proj_b, t_h, t_w, s_h, s_w, num_heads):
    from concourse.bass_utils import run_bass_kernel_spmd

    nc = _get_program()
    in_maps = _prep_in_maps(x_v, x_i, qkv_w, proj_w, proj_b)
    res = run_bass_kernel_spmd(nc, in_maps, list(range(N_CORES)))
    return _decode_out(res)


# revision 46
# speedup vs baseline: 1.6551x; 1.0083x over previous
"""Trainium2 Bass kernel for the two-template sparse cross-modal attention module.

Sharding: data-parallel over batch B=32 across 8 NeuronCores (4 samples/core).
Each sample carries two modality streams (v, i) that must be co-resident
because search tokens attend to the template keys of BOTH modalities.

All matmul operands are bf16 (1 PE cycle/row regardless of free size); all
data I/O is bf16 with host-side conversion.

Per-core program (per sample s, streams st in {v, i}):
  A. QK^T in transposed layout: QKT[1536, 384] = qkv_w[0:1536] @ x.T
     (per-head Q.T, K.T as [64, tok] partition rows).
  B. V in natural layout [tok, 768] with a ones column per head
     ([tok, 65]) so the AV matmul also emits the softmax denominator l
     as a per-partition column.
  C. Attention per head with scores transposed (S.T[k, q] = K Q.T) but AV
     in NATURAL layout: o[q, 65] = sum_k es[k, q].T-matmul v1[k, 65].
     Moving dim of AV is 65 (vs 384 transposed) -> half the PE rows.  The
     denominator lands per-partition, so normalization is one [128,1]
     reciprocal + one broadcast tensor_mul per head pair (no partition
     broadcast).  Scores per head pack into 3 PSUM banks; exp runs as 5
     Act instructions per head pair ([384]x2, [512]x3).
  D. O natural -> O.T via PE transpose (identity matmul, [128,128] tiles).
  E. Transposed projection Y.T[cout, tok] = projw.T-chunks @ O.T with the
     bias folded into the PSUM->SBUF copy as a per-partition tensor_scalar
     add.  Y.T is DMAed out in bf16; the host transposes back.

Emission is software-pipelined: attention of sample s (Act/exp-bound) is
interleaved in program order with its own transpose+projection (as soon as
each head pair is normalized) and with phases A/B of sample s+1 (PE-bound),
so neither the PE nor the Act engine waits on the other.  Weight DMAs are
split so the first matmul starts after ~2 transfers.
"""

import numpy as np

for _p in ("/opt/trn_rl_repo", "/root/.axon_site/_ro/trn_rl_repo"):
    import os
    import sys

    if os.path.isdir(_p) and _p not in sys.path:
        sys.path.append(_p)

B = 32
N_CORES = 8
SAMPLES = 4  # per core
C = 768
NTOK = 384
H = 12
DH = 64
MT = 128  # template tokens
CCH = C // 128  # 6 contraction chunks
MCH = 12  # QK row chunks (1536/128)
TCH = NTOK // 128  # 3 token chunks
WSCALE = 64.0
SCALE = DH ** (-0.5)
SSCALE = SCALE / (WSCALE * WSCALE)

_PROG_CACHE = {}


def _build_program():
    import concourse.bass as bass  # noqa: F401
    import concourse.tile as tile
    from concourse import bacc, mybir
    from concourse.masks import make_identity

    f32 = mybir.dt.float32
    bf16 = mybir.dt.bfloat16
    fp8 = mybir.dt.float8e4
    DR = mybir.MatmulPerfMode.DoubleRow
    Act = mybir.ActivationFunctionType

    nc = bacc.Bacc(None, target_bir_lowering=False)
    _lp = nc.allow_low_precision(reason="fp8/bf16 matmul inputs, fp32 PSUM accumulation")
    _lp.__enter__()

    # x and qkv_w ship as error-compensated fp8 pairs (hi + residual, both
    # e4m3, weights pre-scaled by WSCALE): three DoubleRow matmuls
    # (hi*hi + hi*lo + lo*hi) give bf16-grade accuracy at 2x the fp8 rate.
    # The 64x weight scale rides through scores (folded into the exp scale)
    # and through V (the denominator ones-column is 64 as well).
    xt_d = nc.dram_tensor(
        "xt", [2 * SAMPLES, 128, 2, CCH, NTOK], fp8, kind="ExternalInput"
    )
    # piece-major: piece j holds output-columns j*384..(j+1)*384, contiguous
    # per partition so each DMA descriptor is one 4.6KB run
    qkvw_d = nc.dram_tensor(
        "qkvwT", [CCH, 128, 2, CCH, 384], fp8, kind="ExternalInput"
    )
    projw_d = nc.dram_tensor("projwT", [128, CCH, C], bf16, kind="ExternalInput")
    bias_d = nc.dram_tensor("bias", [128, CCH], f32, kind="ExternalInput")
    y_d = nc.dram_tensor("y", [2 * SAMPLES, 128, CCH, NTOK], bf16, kind="ExternalOutput")

    with tile.TileContext(nc) as tc:
        with (
            tc.tile_pool(name="consts", bufs=1) as consts,
            tc.tile_pool(name="xtp", bufs=2) as xtp,
            tc.tile_pool(name="qktp", bufs=2) as qktp,
            tc.tile_pool(name="v1p", bufs=2) as v1p,
            tc.tile_pool(name="osbp", bufs=2) as osbp,
            tc.tile_pool(name="otTp", bufs=2) as otTp,
            tc.tile_pool(name="yp", bufs=2) as yp,
            tc.tile_pool(name="esp", bufs=4) as esp,
            tc.tile_pool(name="rp", bufs=4) as rp,
            tc.tile_pool(name="pap", bufs=3, space="PSUM") as pap,
            tc.tile_pool(name="pscap", bufs=1, space="PSUM") as pscap,
            tc.tile_pool(name="pscbp", bufs=1, space="PSUM") as pscbp,
            tc.tile_pool(name="psccp", bufs=1, space="PSUM") as psccp,
            tc.tile_pool(name="popp", bufs=2, space="PSUM") as popp,
        ):
            qkvw_sb = consts.tile([128, 2, CCH, 3 * C], fp8, name="qkvw")
            projw_sb = consts.tile([128, CCH, C], bf16)
            bias_sb = consts.tile([128, CCH], f32)
            ident = consts.tile([128, 128], bf16)
            make_identity(nc, ident)

            xt_t = [None] * SAMPLES
            qkt_t = [None] * SAMPLES
            v1_t = [None] * SAMPLES
            osb_t = [None] * SAMPLES
            otT_t = [None] * SAMPLES
            y_t = [[None, None] for _ in range(SAMPLES)]
            acopy_ctr = [0]  # round-robin counter for A-copy engine split

            def dma_const_units():
                # qkv weights split per 384-column piece and hi/lo half so
                # phase A can start after the first small transfers
                for j in range(CCH):
                    for p in range(2):
                        yield lambda j=j, p=p: nc.sync.dma_start(
                            out=qkvw_sb[:, p, :, j * 384 : (j + 1) * 384],
                            in_=qkvw_d[j, :, p],
                        )
                yield lambda: nc.sync.dma_start(out=projw_sb, in_=projw_d[:, :, :])
                yield lambda: nc.sync.dma_start(out=bias_sb, in_=bias_d[:, :])

            def dma_in_units(s):
                xt_t[s] = xtp.tile(
                    [128, 2, CCH, 2, NTOK], fp8, tag="xt", name=f"xt_{s}"
                )
                for st in range(2):
                    for p in range(2):
                        yield lambda st=st, p=p: nc.sync.dma_start(
                            out=xt_t[s][:, p, :, st, :],
                            in_=xt_d[2 * s + st, :, p],
                        )

            def psum_copy(out, in_):
                # GPSIMD cannot touch PSUM on hardware: split the PSUM->SBUF
                # copies between the Act engine (1 in 6) and the DVE
                if acopy_ctr[0] % 6 == 0:
                    nc.scalar.activation(out, in_, Act.Copy)
                else:
                    nc.vector.tensor_copy(out=out, in_=in_)
                acopy_ctr[0] += 1

            # compensated-fp8 DoubleRow contraction: hi*hi + hi*lo + lo*hi
            HL = ((0, 0), (0, 1), (1, 0))

            def a_unit(s, st, m):
                pq = pap.tile([128, NTOK], f32, tag="pa", name=f"pa_a{s}_{st}_{m}")
                for wp, xp in HL:
                    for c2 in range(CCH // 2):
                        nc.tensor.matmul(
                            pq,
                            qkvw_sb[:, wp, 2 * c2 : 2 * c2 + 2, m * 128 : (m + 1) * 128],
                            xt_t[s][:, xp, 2 * c2 : 2 * c2 + 2, st, :],
                            start=((wp, xp) == HL[0] and c2 == 0),
                            stop=((wp, xp) == HL[-1] and c2 == CCH // 2 - 1),
                            perf_mode=DR,
                        )
                psum_copy(qkt_t[s][:, m, st, :], pq)

            def b_unit(s, st, t, n):
                pv = pap.tile([128, NTOK], f32, tag="pa", name=f"pa_b{s}_{st}_{t}_{n}")
                for xp, wp in HL:
                    for c2 in range(CCH // 2):
                        nc.tensor.matmul(
                            pv,
                            xt_t[s][
                                :, xp, 2 * c2 : 2 * c2 + 2, st, t * 128 : (t + 1) * 128
                            ],
                            qkvw_sb[
                                :, wp, 2 * c2 : 2 * c2 + 2,
                                2 * C + n * NTOK : 2 * C + (n + 1) * NTOK,
                            ],
                            start=((xp, wp) == HL[0] and c2 == 0),
                            stop=((xp, wp) == HL[-1] and c2 == CCH // 2 - 1),
                            perf_mode=DR,
                        )
                psum_copy(
                    v1_t[s][:, t, st, 6 * n : 6 * n + 6, 0:64],
                    pv.rearrange("p (h d) -> p h d", h=6),
                )
                if t == 0 and n == 0:
                    # 64 = WSCALE: the AV denominator column must carry the
                    # same scale as the (pre-scaled) V values
                    nc.vector.memset(v1_t[s][:, :, st, :, 64:65], 64.0)

            def ab_units(s):
                """Phase A (QK^T transposed) + phase B (V natural) for sample s,
                ordered so attention pair (st, hp) is ready as early as possible:
                A chunks hp-major (q then k, both streams), B chunks n-major."""
                qkt_t[s] = qktp.tile(
                    [128, MCH, 2, NTOK], bf16, tag="qkt", name=f"qkt_{s}"
                )
                v1_t[s] = v1p.tile(
                    [128, TCH, 2, H, 65], bf16, tag="v1", name=f"v1_{s}"
                )
                for st in range(2):
                    for m in range(MCH):
                        yield lambda st=st, m=m: a_unit(s, st, m)
                for st in range(2):
                    for n in range(2):
                        for t in range(TCH):
                            yield lambda st=st, t=t, n=n: b_unit(s, st, t, n)

            # Attention is emitted as a one-head software pipeline: the S
            # matmuls + exps of head h+1 are interleaved with the AV matmuls
            # of head h, so the PE->Act->PE loop of a single head never sits
            # on the critical path and the Act queue stays continuously fed.
            ht = {}  # (s, st, h) -> dict of live tiles

            def s_ab(s, st, h):
                """Scores for the own-mt (slot 0, exp A) and own-search
                (slot 1, exp B) keys, plus their exps."""
                qkt = qkt_t[s]
                # separate tiles per score slot: dependency tracking is
                # tile-granular, so a shared tile would serialize the next
                # head's score matmuls behind ALL of this head's exps
                pscA = pscap.tile([128, NTOK], f32, tag="pscA", name=f"pscA_{s}_{st}_{h}")
                pscB = pscbp.tile([128, 512], f32, tag="pscB", name=f"pscB_{s}_{st}_{h}")
                es = esp.tile([128, 3, 512], bf16, tag="es", name=f"es_{s}_{st}_{h}")
                ht[(s, st, h)] = {"pscA": pscA, "pscB": pscB, "es": es}
                ro = (h % 2) * 64
                hp = h // 2
                qT = qkt[ro : ro + 64, hp, st, :]
                kT = qkt[ro : ro + 64, 6 + hp, st, :]
                nc.tensor.matmul(pscA, kT[:, 0:MT], qT)
                nc.tensor.matmul(pscB[:, 0:256], kT[:, MT : MT + 128], qT[:, MT:])
                nc.tensor.matmul(pscB[:, 256:512], kT[:, MT + 128 :], qT[:, MT:])
                nc.scalar.activation(es[:, 0, 0:NTOK], pscA, Act.Exp, scale=SSCALE)
                nc.scalar.activation(es[:, 1, :], pscB, Act.Exp, scale=SSCALE)

            def s_c(s, st, h):
                """Scores for the other-modality template keys (slot 2) + exp."""
                qkt = qkt_t[s]
                t = ht[(s, st, h)]
                pscC = psccp.tile([128, 256], f32, tag="pscC", name=f"pscC_{s}_{st}_{h}")
                ro = (h % 2) * 64
                hp = h // 2
                qT = qkt[ro : ro + 64, hp, st, :]
                kTo = qkt[ro : ro + 64, 6 + hp, 1 - st, :]
                nc.tensor.matmul(pscC, kTo[:, 0:MT], qT[:, MT:])
                nc.scalar.activation(
                    t["es"][:, 2, 0:256], pscC, Act.Exp, scale=SSCALE
                )

            def av_full(s, st, h):
                """All AV matmuls for head h.  Runs a full pipeline period
                after the head's exps, so nothing here waits on the Act
                engine.  Each search q-chunk's accumulation group runs to
                completion before the next opens: start_tensor_calc lazily
                zeroes the whole PSUM tile, so interleaving open groups in
                one tile destroys the earlier group's partial sums.
                """
                v1 = v1_t[s]
                t = ht[(s, st, h)]
                es = t["es"]
                # 96-f32 stride keeps every matmul PSUM dst 16B-aligned
                po = popp.tile([128, TCH, 96], f32, tag="po", name=f"po_{s}_{st}_{h}")
                t["po"] = po
                # mt queries: attend own-mt keys only (closed group)
                nc.tensor.matmul(po[:, 0, 0:65], es[:, 0, 0:MT], v1[:, 0, st, h, :])
                for u in (1, 2):
                    qo = (u - 1) * 128
                    dst = po[:, u, 0:65]
                    nc.tensor.matmul(
                        dst, es[:, 0, MT + qo : MT + qo + 128],
                        v1[:, 0, st, h, :], start=True, stop=False,
                    )
                    nc.tensor.matmul(
                        dst, es[:, 1, qo : qo + 128],
                        v1[:, 1, st, h, :], start=False, stop=False,
                    )
                    nc.tensor.matmul(
                        dst, es[:, 1, 256 + qo : 256 + qo + 128],
                        v1[:, 2, st, h, :], start=False, stop=False,
                    )
                    nc.tensor.matmul(
                        dst, es[:, 2, qo : qo + 128],
                        v1[:, 0, 1 - st, h, :], start=False, stop=True,
                    )

            def av_finish(s, st, h):
                """Reciprocal of the denominator column + broadcast normalize."""
                if osb_t[s] is None:
                    osb_t[s] = osbp.tile(
                        [128, TCH, 2, C], bf16, tag="osb", name=f"osb_{s}"
                    )
                po = ht.pop((s, st, h))["po"]
                rl = rp.tile([128, TCH], f32, tag="rl", name=f"rl_{s}_{st}_{h}")
                nc.vector.reciprocal(out=rl, in_=po[:, :, 64:65])
                nc.vector.tensor_mul(
                    osb_t[s][:, :, st, h * 64 : (h + 1) * 64],
                    po[:, :, 0:64],
                    rl[:, :, None].broadcast_to([128, TCH, 64]),
                )

            def d_unit(s, st, cc):
                if otT_t[s] is None:
                    otT_t[s] = otTp.tile(
                        [128, CCH, 2, NTOK], bf16, tag="otT", name=f"otT_{s}"
                    )
                pt = pap.tile([128, NTOK], bf16, tag="pa", name=f"pa_d{s}_{st}_{cc}")
                for u in range(TCH):
                    nc.tensor.transpose(
                        pt[:, u * 128 : (u + 1) * 128],
                        osb_t[s][:, u, st, cc * 128 : (cc + 1) * 128],
                        ident,
                    )
                nc.vector.tensor_copy(out=otT_t[s][:, cc, st, :], in_=pt)

            def e_unit(s, st, m2):
                if y_t[s][st] is None:
                    y_t[s][st] = yp.tile(
                        [128, CCH, NTOK], bf16, tag="y", name=f"y_{s}_{st}"
                    )
                py = pap.tile([128, NTOK], f32, tag="pa", name=f"pa_e{s}_{st}_{m2}")
                for c in range(CCH):
                    nc.tensor.matmul(
                        py,
                        projw_sb[:, c, m2 * 128 : (m2 + 1) * 128],
                        otT_t[s][:, c, st, :],
                        start=(c == 0),
                        stop=(c == CCH - 1),
                    )
                nc.vector.tensor_scalar_add(
                    y_t[s][st][:, m2, :], py, bias_sb[:, m2 : m2 + 1]
                )
                # two half-DMAs per stream so the last one drains faster
                if m2 == CCH // 2 - 1 or m2 == CCH - 1:
                    half = m2 // (CCH // 2)
                    sl = slice(half * (CCH // 2), (half + 1) * (CCH // 2))
                    nc.sync.dma_start(
                        out=y_d[2 * s + st, :, sl, :], in_=y_t[s][st][:, sl, :]
                    )

            def de_units(s):
                for st in range(2):
                    for cc in range(CCH):
                        yield lambda st=st, cc=cc: d_unit(s, st, cc)
                    for m2 in range(CCH):
                        yield lambda st=st, m2=m2: e_unit(s, st, m2)

            # ---- software-pipelined emission ----
            # One continuous stream: attention heads of every sample in
            # sequence, with a single global filler queue (phases A/B of the
            # next sample, D/E of the current one as their inputs retire).
            # Filler is consumed at splice points inside each head, paced by
            # estimated PE time, and spills across sample boundaries.
            from collections import deque

            fill_q = deque()
            spent = [0.0]  # estimated PE-ns of filler consumed

            def splice_upto(tgt_ns):
                while fill_q and spent[0] < tgt_ns:
                    run_one()

            consts_dma = list(dma_const_units())
            first_in = list(dma_in_units(0))
            # DMA order: stream-0 x and its first weight pieces leapfrog so
            # the first matmuls start ~3.5us in; stream-1 x follows while
            # stream-0's phase A computes
            order = [first_in[0], consts_dma[0], first_in[1], consts_dma[1],
                     consts_dma[2], consts_dma[3], consts_dma[4],
                     consts_dma[5], consts_dma[6], consts_dma[7],
                     first_in[2], first_in[3]]
            for u in order:
                u()
            for u in consts_dma[8:]:
                u()
            for u in ab_units(0):
                u()

            # filler pacing: per-head PE-ns of filler, slightly below the
            # production rate so a backlog accumulates for the last sample
            PER_HEAD = 1400.0
            tgt_base = [0.0]
            appended = [0]  # items ever appended to fill_q
            ran = [0]  # items ever consumed

            def run_one():
                cost, u, then = fill_q.popleft()
                u()
                ran[0] += 1
                spent[0] += cost
                if then:
                    for item in then:
                        fill_q.append(item)
                        appended[0] += 1

            def push(cost, u, then=None):
                fill_q.append((cost, u, then))
                appended[0] += 1

            def after_finish(s, st, h):
                if h % 2 == 1:
                    hp = h // 2
                    then = None
                    if hp == 5:
                        then = [
                            (960.0,
                             (lambda s=s, st=st, m2=m2: e_unit(s, st, m2)),
                             None)
                            for m2 in range(CCH)
                        ]
                    push(200.0, (lambda s=s, st=st, hp=hp: d_unit(s, st, hp)),
                         then)

            markers = {}
            flat = [
                (s, st, h)
                for s in range(SAMPLES)
                for st in range(2)
                for h in range(H)
            ]
            per_head = [PER_HEAD]
            prev = [None]

            def step(cur):
                """One pipeline step: AV of the previous head wrapped around
                S+exp of the current one, filler spliced at the two points
                where the PE would otherwise wait on the Act engine."""
                s, st, h = cur
                if st == 0 and h == 0:
                    if s + 1 < SAMPLES:
                        for u in dma_in_units(s + 1):
                            push(0.0, u)
                        for u in ab_units(s + 1):
                            push(720.0, u)
                        # A/B of s+1 must be fully emitted before the first
                        # head of s+1 (the in-order PE queue would otherwise
                        # invert the qkt/v1 dependencies)
                        markers[s + 1] = appended[0]
                    else:
                        # final sample: spread the backlog + its own D/E
                        # evenly over the remaining heads
                        left = sum(c for c, _, _ in fill_q) + 2 * H * 600.0
                        per_head[0] = left / (2 * H)
                    if s > 0:
                        while ran[0] < markers[s]:
                            run_one()
                p = prev[0]
                if p is not None:
                    # AV of the previous head: all three exps it needs
                    # completed during the previous period, so none of these
                    # matmuls ever wait on the Act engine
                    av_full(*p)
                s_ab(s, st, h)
                s_c(s, st, h)
                splice_upto(tgt_base[0] + 0.8 * per_head[0])
                if p is not None:
                    av_finish(*p)
                    after_finish(*p)
                splice_upto(tgt_base[0] + per_head[0])
                tgt_base[0] += per_head[0]
                prev[0] = cur

            for cur in flat:
                step(cur)
            av_full(*prev[0])
            av_finish(*prev[0])
            after_finish(*prev[0])
            while fill_q:
                run_one()

    _lp.__exit__(None, None, None)
    nc.compile()
    return nc


def _get_program():
    if "prog" not in _PROG_CACHE:
        _PROG_CACHE["prog"] = _build_program()
    return _PROG_CACHE["prog"]


def _to_bf16(a):
    import ml_dtypes

    return np.ascontiguousarray(a.astype(ml_dtypes.bfloat16))


def _to_fp8_pair(a):
    import ml_dtypes

    f8 = ml_dtypes.float8_e4m3
    hi = a.astype(f8)
    lo = (a - hi.astype(np.float32)).astype(f8)
    return np.ascontiguousarray(hi), np.ascontiguousarray(lo)


def _prep_in_maps(x_v, x_i, qkv_w, proj_w, proj_b):
    # weights: [out, in] -> transposed [in, out] -> [128, CCH, out] chunked
    qkvwT = np.asarray(qkv_w, np.float32).T.reshape(CCH, 128, 3 * C).transpose(1, 0, 2)
    projwT = np.asarray(proj_w, np.float32).T.reshape(CCH, 128, C).transpose(1, 0, 2)
    bias = np.ascontiguousarray(
        np.asarray(proj_b, np.float32).reshape(CCH, 128).T
    )
    qkvwTh, qkvwTl = _to_fp8_pair(qkvwT * WSCALE)
    qkvwT8 = np.stack([qkvwTh, qkvwTl], axis=1)  # [128, 2, CCH, 3C]
    qkvwT8 = np.ascontiguousarray(
        qkvwT8.reshape(128, 2, CCH, CCH, 384).transpose(3, 0, 1, 2, 4)
    )
    projwT = _to_bf16(projwT)
    in_maps = []
    for core in range(N_CORES):
        sl = slice(core * SAMPLES, (core + 1) * SAMPLES)
        # streams interleaved: 2s = v-sample, 2s+1 = i-sample;
        # layout [128, CCH, NTOK]: partition p, chunk c -> channel c*128+p
        xs = np.empty((2 * SAMPLES, 128, CCH, NTOK), np.float32)
        xs[0::2] = (
            np.asarray(x_v[sl], np.float32)
            .transpose(0, 2, 1)
            .reshape(SAMPLES, CCH, 128, NTOK)
            .transpose(0, 2, 1, 3)
        )
        xs[1::2] = (
            np.asarray(x_i[sl], np.float32)
            .transpose(0, 2, 1)
            .reshape(SAMPLES, CCH, 128, NTOK)
            .transpose(0, 2, 1, 3)
        )
        xth, xtl = _to_fp8_pair(xs)
        xt8 = np.ascontiguousarray(np.stack([xth, xtl], axis=2))
        in_maps.append(
            {
                "xt": xt8,
                "qkvwT": qkvwT8,
                "projwT": projwT,
                "bias": bias,
            }
        )
    return in_maps


def _decode_out(res):
    out_v = np.empty((B, NTOK, C), np.float32)
    out_i = np.empty((B, NTOK, C), np.float32)
    for core in range(N_CORES):
        y = np.asarray(res.results[core]["y"], dtype=np.float32)
        # [2S, 128, CCH, NTOK] -> [2S, CCH*128 = C, NTOK] -> [2S, NTOK, C]
        y = y.transpose(0, 2, 1, 3).reshape(2 * SAMPLES, C, NTOK).transpose(0, 2, 1)
        sl = slice(core * SAMPLES, (core + 1) * SAMPLES)
        out_v[sl] = y[0::2]
        out_i[sl] = y[1::2]
    return out_v, out_i


def kernel(x_v, x_i, qkv_w, proj_w, proj_b, t_h, t_w, s_h, s_w, num_heads):
    from concourse.bass_utils import run_bass_kernel_spmd

    nc = _get_program()
    in_maps = _prep_in_maps(x_v, x_i, qkv_w, proj_w, proj_b)
    res = run_bass_kernel_spmd(nc, in_maps, list(range(N_CORES)))
    return _decode_out(res)


# revision 49
# speedup vs baseline: 1.7892x; 1.0810x over previous
"""Trainium2 Bass kernel for the two-template sparse cross-modal attention module.

Sharding: data-parallel over batch B=32 across 8 NeuronCores (4 samples/core).
Each sample carries two modality streams (v, i) that must be co-resident
because search tokens attend to the template keys of BOTH modalities.

All matmul operands are bf16 (1 PE cycle/row regardless of free size); all
data I/O is bf16 with host-side conversion.

Per-core program (per sample s, streams st in {v, i}):
  A. QK^T in transposed layout: QKT[1536, 384] = qkv_w[0:1536] @ x.T
     (per-head Q.T, K.T as [64, tok] partition rows).
  B. V in natural layout [tok, 768] with a ones column per head
     ([tok, 65]) so the AV matmul also emits the softmax denominator l
     as a per-partition column.
  C. Attention per head with scores transposed (S.T[k, q] = K Q.T) but AV
     in NATURAL layout: o[q, 65] = sum_k es[k, q].T-matmul v1[k, 65].
     Moving dim of AV is 65 (vs 384 transposed) -> half the PE rows.  The
     denominator lands per-partition, so normalization is one [128,1]
     reciprocal + one broadcast tensor_mul per head pair (no partition
     broadcast).  Scores per head pack into 3 PSUM banks; exp runs as 5
     Act instructions per head pair ([384]x2, [512]x3).
  D. O natural -> O.T via PE transpose (identity matmul, [128,128] tiles).
  E. Transposed projection Y.T[cout, tok] = projw.T-chunks @ O.T with the
     bias folded into the PSUM->SBUF copy as a per-partition tensor_scalar
     add.  Y.T is DMAed out in bf16; the host transposes back.

Emission is software-pipelined: attention of sample s (Act/exp-bound) is
interleaved in program order with its own transpose+projection (as soon as
each head pair is normalized) and with phases A/B of sample s+1 (PE-bound),
so neither the PE nor the Act engine waits on the other.  Weight DMAs are
split so the first matmul starts after ~2 transfers.
"""

import numpy as np

for _p in ("/opt/trn_rl_repo", "/root/.axon_site/_ro/trn_rl_repo"):
    import os
    import sys

    if os.path.isdir(_p) and _p not in sys.path:
        sys.path.append(_p)

B = 32
N_CORES = 8
SAMPLES = 4  # per core
C = 768
NTOK = 384
H = 12
DH = 64
MT = 128  # template tokens
CCH = C // 128  # 6 contraction chunks
MCH = 12  # QK row chunks (1536/128)
TCH = NTOK // 128  # 3 token chunks
WSCALE = 64.0
SCALE = DH ** (-0.5)
SSCALE = SCALE / (WSCALE * WSCALE)

_PROG_CACHE = {}


def _build_program():
    import concourse.bass as bass  # noqa: F401
    import concourse.tile as tile
    from concourse import bacc, mybir
    from concourse.masks import make_identity

    f32 = mybir.dt.float32
    bf16 = mybir.dt.bfloat16
    fp8 = mybir.dt.float8e4
    DR = mybir.MatmulPerfMode.DoubleRow
    Act = mybir.ActivationFunctionType

    nc = bacc.Bacc(None, target_bir_lowering=False)
    _lp = nc.allow_low_precision(reason="fp8/bf16 matmul inputs, fp32 PSUM accumulation")
    _lp.__enter__()

    # x and qkv_w ship as error-compensated fp8 pairs (hi + residual, both
    # e4m3, weights pre-scaled by WSCALE): three DoubleRow matmuls
    # (hi*hi + hi*lo + lo*hi) give bf16-grade accuracy at 2x the fp8 rate.
    # The 64x weight scale rides through scores (folded into the exp scale)
    # and through V (the denominator ones-column is 64 as well).
    xt_d = nc.dram_tensor(
        "xt", [2 * SAMPLES, 128, 2, CCH, NTOK], fp8, kind="ExternalInput"
    )
    # piece-major: piece j holds output-columns j*384..(j+1)*384, contiguous
    # per partition so each DMA descriptor is one 4.6KB run
    qkvw_d = nc.dram_tensor(
        "qkvwT", [CCH, 128, 2, CCH, 384], fp8, kind="ExternalInput"
    )
    projw_d = nc.dram_tensor("projwT", [128, CCH, C], bf16, kind="ExternalInput")
    bias_d = nc.dram_tensor("bias", [128, CCH], f32, kind="ExternalInput")
    y_d = nc.dram_tensor("y", [2 * SAMPLES, 128, CCH, NTOK], bf16, kind="ExternalOutput")

    with tile.TileContext(nc) as tc:
        with (
            tc.tile_pool(name="consts", bufs=1) as consts,
            tc.tile_pool(name="xtp", bufs=2) as xtp,
            tc.tile_pool(name="qktp", bufs=2) as qktp,
            tc.tile_pool(name="v1p", bufs=2) as v1p,
            tc.tile_pool(name="osbp", bufs=2) as osbp,
            tc.tile_pool(name="otTp", bufs=2) as otTp,
            tc.tile_pool(name="yp", bufs=2) as yp,
            tc.tile_pool(name="esp", bufs=4) as esp,
            tc.tile_pool(name="rp", bufs=4) as rp,
            tc.tile_pool(name="pap", bufs=3, space="PSUM") as pap,
            tc.tile_pool(name="pscap", bufs=1, space="PSUM") as pscap,
            tc.tile_pool(name="pscbp", bufs=1, space="PSUM") as pscbp,
            tc.tile_pool(name="psccp", bufs=1, space="PSUM") as psccp,
            tc.tile_pool(name="popp", bufs=2, space="PSUM") as popp,
        ):
            qkvw_sb = consts.tile([128, 2, CCH, 3 * C], fp8, name="qkvw")
            projw_sb = consts.tile([128, CCH, C], bf16)
            bias_sb = consts.tile([128, CCH], f32)
            ident = consts.tile([128, 128], bf16)
            make_identity(nc, ident)

            xt_t = [None] * SAMPLES
            qkt_t = [None] * SAMPLES
            v1_t = [None] * SAMPLES
            osb_t = [None] * SAMPLES
            otT_t = [None] * SAMPLES
            y_t = [[None, None] for _ in range(SAMPLES)]
            acopy_ctr = [0]  # round-robin counter for A-copy engine split

            def dma_const_units():
                # qkv weights split per 384-column piece and hi/lo half so
                # phase A can start after the first small transfers
                for j in range(CCH):
                    for p in range(2):
                        yield lambda j=j, p=p: nc.sync.dma_start(
                            out=qkvw_sb[:, p, :, j * 384 : (j + 1) * 384],
                            in_=qkvw_d[j, :, p],
                        )
                yield lambda: nc.sync.dma_start(out=projw_sb, in_=projw_d[:, :, :])
                yield lambda: nc.sync.dma_start(out=bias_sb, in_=bias_d[:, :])

            def dma_in_units(s):
                xt_t[s] = xtp.tile(
                    [128, 2, CCH, 2, NTOK], fp8, tag="xt", name=f"xt_{s}"
                )
                for st in range(2):
                    for p in range(2):
                        yield lambda st=st, p=p: nc.sync.dma_start(
                            out=xt_t[s][:, p, :, st, :],
                            in_=xt_d[2 * s + st, :, p],
                        )

            def psum_copy(out, in_):
                # GPSIMD cannot touch PSUM on hardware: split the PSUM->SBUF
                # copies between the Act engine (1 in 6) and the DVE
                if acopy_ctr[0] % 6 == 0:
                    nc.scalar.activation(out, in_, Act.Copy)
                else:
                    nc.vector.tensor_copy(out=out, in_=in_)
                acopy_ctr[0] += 1

            # compensated-fp8 DoubleRow contraction: hi*hi + hi*lo + lo*hi.
            # Q/K (phase A) drop the x-residual term (score errors average
            # out through the softmax; V errors do not), keeping 2 terms.
            HL = ((0, 0), (0, 1), (1, 0))
            HL_A = ((0, 0), (1, 0))

            def a_unit(s, st, m):
                pq = pap.tile([128, NTOK], f32, tag="pa", name=f"pa_a{s}_{st}_{m}")
                for wp, xp in HL_A:
                    for c2 in range(CCH // 2):
                        nc.tensor.matmul(
                            pq,
                            qkvw_sb[:, wp, 2 * c2 : 2 * c2 + 2, m * 128 : (m + 1) * 128],
                            xt_t[s][:, xp, 2 * c2 : 2 * c2 + 2, st, :],
                            start=((wp, xp) == HL_A[0] and c2 == 0),
                            stop=((wp, xp) == HL_A[-1] and c2 == CCH // 2 - 1),
                            perf_mode=DR,
                        )
                psum_copy(qkt_t[s][:, m, st, :], pq)

            def b_unit(s, st, t, n):
                pv = pap.tile([128, NTOK], f32, tag="pa", name=f"pa_b{s}_{st}_{t}_{n}")
                for xp, wp in HL:
                    for c2 in range(CCH // 2):
                        nc.tensor.matmul(
                            pv,
                            xt_t[s][
                                :, xp, 2 * c2 : 2 * c2 + 2, st, t * 128 : (t + 1) * 128
                            ],
                            qkvw_sb[
                                :, wp, 2 * c2 : 2 * c2 + 2,
                                2 * C + n * NTOK : 2 * C + (n + 1) * NTOK,
                            ],
                            start=((xp, wp) == HL[0] and c2 == 0),
                            stop=((xp, wp) == HL[-1] and c2 == CCH // 2 - 1),
                            perf_mode=DR,
                        )
                psum_copy(
                    v1_t[s][:, t, st, 6 * n : 6 * n + 6, 0:64],
                    pv.rearrange("p (h d) -> p h d", h=6),
                )
                if t == 0 and n == 0:
                    # 64 = WSCALE: the AV denominator column must carry the
                    # same scale as the (pre-scaled) V values
                    nc.vector.memset(v1_t[s][:, :, st, :, 64:65], 64.0)

            def ab_units(s):
                """Phase A (QK^T transposed) + phase B (V natural) for sample s,
                ordered so attention pair (st, hp) is ready as early as possible:
                A chunks hp-major (q then k, both streams), B chunks n-major."""
                qkt_t[s] = qktp.tile(
                    [128, MCH, 2, NTOK], bf16, tag="qkt", name=f"qkt_{s}"
                )
                v1_t[s] = v1p.tile(
                    [128, TCH, 2, H, 65], bf16, tag="v1", name=f"v1_{s}"
                )
                for st in range(2):
                    for m in range(MCH):
                        yield 480.0, (lambda st=st, m=m: a_unit(s, st, m))
                for st in range(2):
                    for n in range(2):
                        for t in range(TCH):
                            yield 720.0, (lambda st=st, t=t, n=n: b_unit(s, st, t, n))

            # Attention is emitted as a one-head software pipeline: the S
            # matmuls + exps of head h+1 are interleaved with the AV matmuls
            # of head h, so the PE->Act->PE loop of a single head never sits
            # on the critical path and the Act queue stays continuously fed.
            ht = {}  # (s, st, h) -> dict of live tiles

            def s_ab(s, st, h):
                """Scores for the own-mt (slot 0, exp A) and own-search
                (slot 1, exp B) keys, plus their exps."""
                qkt = qkt_t[s]
                # separate tiles per score slot: dependency tracking is
                # tile-granular, so a shared tile would serialize the next
                # head's score matmuls behind ALL of this head's exps
                pscA = pscap.tile([128, NTOK], f32, tag="pscA", name=f"pscA_{s}_{st}_{h}")
                pscB = pscbp.tile([128, 512], f32, tag="pscB", name=f"pscB_{s}_{st}_{h}")
                es = esp.tile([128, 3, 512], bf16, tag="es", name=f"es_{s}_{st}_{h}")
                ht[(s, st, h)] = {"pscA": pscA, "pscB": pscB, "es": es}
                ro = (h % 2) * 64
                hp = h // 2
                qT = qkt[ro : ro + 64, hp, st, :]
                kT = qkt[ro : ro + 64, 6 + hp, st, :]
                nc.tensor.matmul(pscA, kT[:, 0:MT], qT)
                nc.tensor.matmul(pscB[:, 0:256], kT[:, MT : MT + 128], qT[:, MT:])
                nc.tensor.matmul(pscB[:, 256:512], kT[:, MT + 128 :], qT[:, MT:])
                nc.scalar.activation(es[:, 0, 0:NTOK], pscA, Act.Exp, scale=SSCALE)
                nc.scalar.activation(es[:, 1, :], pscB, Act.Exp, scale=SSCALE)

            def s_c(s, st, h):
                """Scores for the other-modality template keys (slot 2) + exp."""
                qkt = qkt_t[s]
                t = ht[(s, st, h)]
                pscC = psccp.tile([128, 256], f32, tag="pscC", name=f"pscC_{s}_{st}_{h}")
                ro = (h % 2) * 64
                hp = h // 2
                qT = qkt[ro : ro + 64, hp, st, :]
                kTo = qkt[ro : ro + 64, 6 + hp, 1 - st, :]
                nc.tensor.matmul(pscC, kTo[:, 0:MT], qT[:, MT:])
                nc.scalar.activation(
                    t["es"][:, 2, 0:256], pscC, Act.Exp, scale=SSCALE
                )

            def av_full(s, st, h):
                """All AV matmuls for head h.  Runs a full pipeline period
                after the head's exps, so nothing here waits on the Act
                engine.  Each search q-chunk's accumulation group runs to
                completion before the next opens: start_tensor_calc lazily
                zeroes the whole PSUM tile, so interleaving open groups in
                one tile destroys the earlier group's partial sums.
                """
                v1 = v1_t[s]
                t = ht[(s, st, h)]
                es = t["es"]
                # 96-f32 stride keeps every matmul PSUM dst 16B-aligned
                po = popp.tile([128, TCH, 96], f32, tag="po", name=f"po_{s}_{st}_{h}")
                t["po"] = po
                # mt queries: attend own-mt keys only (closed group)
                nc.tensor.matmul(po[:, 0, 0:65], es[:, 0, 0:MT], v1[:, 0, st, h, :])
                for u in (1, 2):
                    qo = (u - 1) * 128
                    dst = po[:, u, 0:65]
                    nc.tensor.matmul(
                        dst, es[:, 0, MT + qo : MT + qo + 128],
                        v1[:, 0, st, h, :], start=True, stop=False,
                    )
                    nc.tensor.matmul(
                        dst, es[:, 1, qo : qo + 128],
                        v1[:, 1, st, h, :], start=False, stop=False,
                    )
                    nc.tensor.matmul(
                        dst, es[:, 1, 256 + qo : 256 + qo + 128],
                        v1[:, 2, st, h, :], start=False, stop=False,
                    )
                    nc.tensor.matmul(
                        dst, es[:, 2, qo : qo + 128],
                        v1[:, 0, 1 - st, h, :], start=False, stop=True,
                    )

            def av_finish(s, st, h):
                """Reciprocal of the denominator column + broadcast normalize."""
                if osb_t[s] is None:
                    osb_t[s] = osbp.tile(
                        [128, TCH, 2, C], bf16, tag="osb", name=f"osb_{s}"
                    )
                po = ht.pop((s, st, h))["po"]
                rl = rp.tile([128, TCH], f32, tag="rl", name=f"rl_{s}_{st}_{h}")
                nc.vector.reciprocal(out=rl, in_=po[:, :, 64:65])
                nc.vector.tensor_mul(
                    osb_t[s][:, :, st, h * 64 : (h + 1) * 64],
                    po[:, :, 0:64],
                    rl[:, :, None].broadcast_to([128, TCH, 64]),
                )

            def d_unit(s, st, cc):
                if otT_t[s] is None:
                    otT_t[s] = otTp.tile(
                        [128, CCH, 2, NTOK], bf16, tag="otT", name=f"otT_{s}"
                    )
                pt = pap.tile([128, NTOK], bf16, tag="pa", name=f"pa_d{s}_{st}_{cc}")
                for u in range(TCH):
                    nc.tensor.transpose(
                        pt[:, u * 128 : (u + 1) * 128],
                        osb_t[s][:, u, st, cc * 128 : (cc + 1) * 128],
                        ident,
                    )
                nc.vector.tensor_copy(out=otT_t[s][:, cc, st, :], in_=pt)

            def e_unit(s, st, m2):
                if y_t[s][st] is None:
                    y_t[s][st] = yp.tile(
                        [128, CCH, NTOK], bf16, tag="y", name=f"y_{s}_{st}"
                    )
                py = pap.tile([128, NTOK], f32, tag="pa", name=f"pa_e{s}_{st}_{m2}")
                for c in range(CCH):
                    nc.tensor.matmul(
                        py,
                        projw_sb[:, c, m2 * 128 : (m2 + 1) * 128],
                        otT_t[s][:, c, st, :],
                        start=(c == 0),
                        stop=(c == CCH - 1),
                    )
                nc.vector.tensor_scalar_add(
                    y_t[s][st][:, m2, :], py, bias_sb[:, m2 : m2 + 1]
                )
                # two half-DMAs per stream so the last one drains faster
                if m2 == CCH // 2 - 1 or m2 == CCH - 1:
                    half = m2 // (CCH // 2)
                    sl = slice(half * (CCH // 2), (half + 1) * (CCH // 2))
                    nc.sync.dma_start(
                        out=y_d[2 * s + st, :, sl, :], in_=y_t[s][st][:, sl, :]
                    )

            def de_units(s):
                for st in range(2):
                    for cc in range(CCH):
                        yield lambda st=st, cc=cc: d_unit(s, st, cc)
                    for m2 in range(CCH):
                        yield lambda st=st, m2=m2: e_unit(s, st, m2)

            # ---- software-pipelined emission ----
            # One continuous stream: attention heads of every sample in
            # sequence, with a single global filler queue (phases A/B of the
            # next sample, D/E of the current one as their inputs retire).
            # Filler is consumed at splice points inside each head, paced by
            # estimated PE time, and spills across sample boundaries.
            from collections import deque

            fill_q = deque()
            spent = [0.0]  # estimated PE-ns of filler consumed

            def splice_upto(tgt_ns):
                while fill_q and spent[0] < tgt_ns:
                    run_one()

            consts_dma = list(dma_const_units())
            first_in = list(dma_in_units(0))
            # DMA order: stream-0 x and its first weight pieces leapfrog so
            # the first matmuls start ~3.5us in; stream-1 x follows while
            # stream-0's phase A computes
            order = [first_in[0], consts_dma[0], first_in[1], consts_dma[1],
                     consts_dma[2], consts_dma[3], consts_dma[4],
                     consts_dma[5], consts_dma[6], consts_dma[7],
                     first_in[2], first_in[3]]
            for u in order:
                u()
            for u in consts_dma[8:]:
                u()
            for _, u in ab_units(0):
                u()

            # filler pacing: per-head PE-ns of filler, slightly below the
            # production rate so a backlog accumulates for the last sample
            PER_HEAD = 1150.0
            tgt_base = [0.0]
            appended = [0]  # items ever appended to fill_q
            ran = [0]  # items ever consumed

            def run_one():
                cost, u, then = fill_q.popleft()
                u()
                ran[0] += 1
                spent[0] += cost
                if then:
                    for item in then:
                        fill_q.append(item)
                        appended[0] += 1

            def push(cost, u, then=None):
                fill_q.append((cost, u, then))
                appended[0] += 1

            def after_finish(s, st, h):
                if h % 2 == 1:
                    hp = h // 2
                    then = None
                    if hp == 5:
                        then = [
                            (960.0,
                             (lambda s=s, st=st, m2=m2: e_unit(s, st, m2)),
                             None)
                            for m2 in range(CCH)
                        ]
                    push(200.0, (lambda s=s, st=st, hp=hp: d_unit(s, st, hp)),
                         then)

            markers = {}
            flat = [
                (s, st, h)
                for s in range(SAMPLES)
                for st in range(2)
                for h in range(H)
            ]
            per_head = [PER_HEAD]
            prev = [None]

            def step(cur):
                """One pipeline step: AV of the previous head wrapped around
                S+exp of the current one, filler spliced at the two points
                where the PE would otherwise wait on the Act engine."""
                s, st, h = cur
                if st == 0 and h == 0:
                    if s + 1 < SAMPLES:
                        for u in dma_in_units(s + 1):
                            push(0.0, u)
                        for cost, u in ab_units(s + 1):
                            push(cost, u)
                        # A/B of s+1 must be fully emitted before the first
                        # head of s+1 (the in-order PE queue would otherwise
                        # invert the qkt/v1 dependencies)
                        markers[s + 1] = appended[0]
                    else:
                        # final sample: spread the backlog + its own D/E
                        # evenly over the remaining heads
                        left = sum(c for c, _, _ in fill_q) + 2 * H * 600.0
                        per_head[0] = left / (2 * H)
                    if s > 0:
                        while ran[0] < markers[s]:
                            run_one()
                p = prev[0]
                if p is not None:
                    # AV of the previous head: all three exps it needs
                    # completed during the previous period, so none of these
                    # matmuls ever wait on the Act engine
                    av_full(*p)
                s_ab(s, st, h)
                s_c(s, st, h)
                splice_upto(tgt_base[0] + 0.8 * per_head[0])
                if p is not None:
                    av_finish(*p)
                    after_finish(*p)
                splice_upto(tgt_base[0] + per_head[0])
                tgt_base[0] += per_head[0]
                prev[0] = cur

            for cur in flat:
                step(cur)
            av_full(*prev[0])
            av_finish(*prev[0])
            after_finish(*prev[0])
            while fill_q:
                run_one()

    _lp.__exit__(None, None, None)
    nc.compile()
    return nc


def _get_program():
    if "prog" not in _PROG_CACHE:
        _PROG_CACHE["prog"] = _build_program()
    return _PROG_CACHE["prog"]


def _to_bf16(a):
    import ml_dtypes

    return np.ascontiguousarray(a.astype(ml_dtypes.bfloat16))


def _to_fp8_pair(a):
    import ml_dtypes

    f8 = ml_dtypes.float8_e4m3
    hi = a.astype(f8)
    lo = (a - hi.astype(np.float32)).astype(f8)
    return np.ascontiguousarray(hi), np.ascontiguousarray(lo)


def _prep_in_maps(x_v, x_i, qkv_w, proj_w, proj_b):
    # weights: [out, in] -> transposed [in, out] -> [128, CCH, out] chunked
    qkvwT = np.asarray(qkv_w, np.float32).T.reshape(CCH, 128, 3 * C).transpose(1, 0, 2)
    projwT = np.asarray(proj_w, np.float32).T.reshape(CCH, 128, C).transpose(1, 0, 2)
    bias = np.ascontiguousarray(
        np.asarray(proj_b, np.float32).reshape(CCH, 128).T
    )
    qkvwTh, qkvwTl = _to_fp8_pair(qkvwT * WSCALE)
    qkvwT8 = np.stack([qkvwTh, qkvwTl], axis=1)  # [128, 2, CCH, 3C]
    qkvwT8 = np.ascontiguousarray(
        qkvwT8.reshape(128, 2, CCH, CCH, 384).transpose(3, 0, 1, 2, 4)
    )
    projwT = _to_bf16(projwT)
    in_maps = []
    for core in range(N_CORES):
        sl = slice(core * SAMPLES, (core + 1) * SAMPLES)
        # streams interleaved: 2s = v-sample, 2s+1 = i-sample;
        # layout [128, CCH, NTOK]: partition p, chunk c -> channel c*128+p
        xs = np.empty((2 * SAMPLES, 128, CCH, NTOK), np.float32)
        xs[0::2] = (
            np.asarray(x_v[sl], np.float32)
            .transpose(0, 2, 1)
            .reshape(SAMPLES, CCH, 128, NTOK)
            .transpose(0, 2, 1, 3)
        )
        xs[1::2] = (
            np.asarray(x_i[sl], np.float32)
            .transpose(0, 2, 1)
            .reshape(SAMPLES, CCH, 128, NTOK)
            .transpose(0, 2, 1, 3)
        )
        xth, xtl = _to_fp8_pair(xs)
        xt8 = np.ascontiguousarray(np.stack([xth, xtl], axis=2))
        in_maps.append(
            {
                "xt": xt8,
                "qkvwT": qkvwT8,
                "projwT": projwT,
                "bias": bias,
            }
        )
    return in_maps


def _decode_out(res):
    out_v = np.empty((B, NTOK, C), np.float32)
    out_i = np.empty((B, NTOK, C), np.float32)
    for core in range(N_CORES):
        y = np.asarray(res.results[core]["y"], dtype=np.float32)
        # [2S, 128, CCH, NTOK] -> [2S, CCH*128 = C, NTOK] -> [2S, NTOK, C]
        y = y.transpose(0, 2, 1, 3).reshape(2 * SAMPLES, C, NTOK).transpose(0, 2, 1)
        sl = slice(core * SAMPLES, (core + 1) * SAMPLES)
        out_v[sl] = y[0::2]
        out_i[sl] = y[1::2]
    return out_v, out_i


def kernel(x_v, x_i, qkv_w, proj_w, proj_b, t_h, t_w, s_h, s_w, num_heads):
    from concourse.bass_utils import run_bass_kernel_spmd

    nc = _get_program()
    in_maps = _prep_in_maps(x_v, x_i, qkv_w, proj_w, proj_b)
    res = run_bass_kernel_spmd(nc, in_maps, list(range(N_CORES)))
    return _decode_out(res)
